# revision 1
# baseline (speedup 1.0000x reference)
"""Trainium2 Bass kernel for a 3x3 stride-1 pad-1 Conv2d (NCHW).

Problem (hardcoded): x (16, 128, 128, 128) f32, K (3, 3, 128, 256) f32.
The reference reinterprets K's flat buffer as (Cin, kh, kw, Cout) and only
writes output rows/cols 0..124 (the rest of the 128x128 output stays zero).

Strategy: data-parallel over batch — 2 images per NeuronCore on 8 cores.
Per image the padded activation plane (Cin=128 partitions x 130x130) lives
in SBUF; the conv is 9 accumulated matmuls (contraction over Cin=128) per
output tile of 4 rows x 128 cols (N=512, one PSUM bank) per Cout half.
Matmuls run in float32r (TF32-like full-rate PE path; host pre-rounds the
inputs). Only the valid 125x125 region is DMA'd out; the host zeroes the
border strips.
"""

import numpy as np

import concourse.bacc as bacc
import concourse.mybir as mybir
import concourse.tile as tile
from concourse.bass_utils import run_bass_kernel_spmd

N_CORES = 8
B, CIN, H, W = 16, 128, 128, 128
COUT = 256
BPC = B // N_CORES  # images per core
HP, WP = H + 2, W + 2  # zero-padded plane
VALID = 125  # valid output rows/cols; rest is zero
ROWS_PER_TILE = 4  # 4 rows x 128 cols = 512 = one PSUM bank
F32 = mybir.dt.float32
F32R = mybir.dt.float32r

_NC_CACHE = {}


def _build_nc(reps=1):
    nc = bacc.Bacc()
    # Inputs are declared float32r (TF32-like: fp32 with 11 mantissa bits,
    # low 12 bits zero). Host pre-rounds, so DMA'd bytes are valid fp32r.
    # x arrives pre-padded (130x130, zero borders): fp32r is an opt-in ISA
    # dtype that DVE memset doesn't support, so padding happens on host.
    x_in = nc.dram_tensor("x", [BPC, CIN, HP, WP], F32R, kind="ExternalInput")
    w_in = nc.dram_tensor("w", [CIN, 9 * COUT], F32R, kind="ExternalInput")
    out_t = nc.dram_tensor("out", [BPC, COUT, H, W], F32, kind="ExternalOutput")

    with tile.TileContext(nc) as tc:
        with (
            tc.tile_pool(name="wpool", bufs=1) as wpool,
            tc.tile_pool(name="xpool", bufs=2) as xpool,
            tc.tile_pool(name="opool", bufs=6) as opool,
            tc.tile_pool(name="pspool", bufs=8, space="PSUM") as pspool,
        ):
            w_sb = wpool.tile([CIN, 9 * COUT], F32R)
            nc.sync.dma_start(out=w_sb[:], in_=w_in[:])

            for b in [b for _ in range(reps) for b in range(BPC)]:
                x_pad = xpool.tile([CIN, HP, WP], F32R)
                nc.sync.dma_start(out=x_pad[:], in_=x_in[b])

                for rb in range(H // ROWS_PER_TILE):
                    r = rb * ROWS_PER_TILE
                    vr = min(ROWS_PER_TILE, VALID - r)
                    if vr <= 0:
                        continue
                    for c2 in range(2):
                        ps = pspool.tile([128, ROWS_PER_TILE, W], F32)
                        for i, t in enumerate(range(9)):
                            kh, kw = divmod(t, 3)
                            c0 = t * COUT + c2 * 128
                            lhsT = w_sb[:, c0 : c0 + 128]
                            rhs = x_pad[:, r + kh : r + kh + ROWS_PER_TILE, kw : kw + W]
                            nc.tensor.matmul(
                                ps[:],
                                lhsT,
                                rhs,
                                start=(i == 0),
                                stop=(i == 8),
                            )
                        ob = opool.tile([128, ROWS_PER_TILE, W], F32)
                        nc.vector.tensor_copy(out=ob[:], in_=ps[:])
                        nc.sync.dma_start(
                            out=out_t[b, c2 * 128 : (c2 + 1) * 128, r : r + vr, 0:VALID],
                            in_=ob[:, 0:vr, 0:VALID],
                        )
    # Bacc defers register allocation and wait-splitting to compile(),
    # which finalize() runs; the SPMD exec path expects it done already.
    nc.finalize()
    return nc


def _get_nc(reps=1):
    if reps not in _NC_CACHE:
        _NC_CACHE[reps] = _build_nc(reps)
    return _NC_CACHE[reps]


def _round_fp32r(a):
    """Round fp32 to the hardware fp32r format: 11 mantissa bits, RNE."""
    u = np.ascontiguousarray(a, dtype=np.float32).view(np.uint32)
    r = (u + np.uint32(0x7FF) + ((u >> np.uint32(12)) & np.uint32(1))) & np.uint32(
        0xFFFFF000
    )
    return r.view(np.float32)


def _run(x, K, trace=False, reps=1):
    x_pad = np.zeros((B, CIN, HP, WP), dtype=np.float32)
    x_pad[:, :, 1 : H + 1, 1 : W + 1] = _round_fp32r(x)
    # Reference reinterprets K's flat buffer as (Cin, kh, kw, Cout); flat
    # (128, 2304) rows are Cin, cols are (kh*3+kw)*256 + cout.
    w_host = _round_fp32r(np.asarray(K, dtype=np.float32)).reshape(CIN, 9 * COUT)
    in_maps = [
        {"x": x_pad[i * BPC : (i + 1) * BPC], "w": w_host} for i in range(N_CORES)
    ]
    res = run_bass_kernel_spmd(
        _get_nc(reps), in_maps, list(range(N_CORES)), trace=trace
    )
    out = np.concatenate([res.results[i]["out"] for i in range(N_CORES)], axis=0)
    # Device only writes the valid 125x125 region; zero the border strips.
    out[:, :, VALID:, :] = 0
    out[:, :, :, VALID:] = 0
    return out, res


def kernel(x, K):
    out, _ = _run(x, K, trace=False)
    return out



# revision 3
# speedup vs baseline: 1.2046x; 1.2046x over previous
"""Trainium2 Bass kernel for a 3x3 stride-1 pad-1 Conv2d (NCHW).

Problem (hardcoded): x (16, 128, 128, 128) f32, K (3, 3, 128, 256) f32.
The reference reinterprets K's flat buffer as (Cin, kh, kw, Cout) and only
writes output rows/cols 0..124 (the rest of the 128x128 output stays zero).

Strategy: data-parallel over batch — 2 images per NeuronCore on 8 cores.
The activation plane is zero-padded to 130 rows x 128 COLS (cols -1..126;
original col 127 only feeds invalid output cols >= 126, so it's dropped).
With plane row stride == output row stride == 128, each tap's rhs for a
4-output-row tile is ONE contiguous 510-element run, and the matmul's
PSUM output lands exactly in [4, 128] row-major layout — no strided APs
anywhere on the matmul path. Each tile accumulates 9 taps (contraction
over Cin=128 partitions) into one PSUM bank per Cout half, in float32r
(TF32-like full-rate PE path; host pre-rounds inputs). The plane streams
in row-band chunks (first chunk is one row-tile so the PE starts ~6us in).
Stores ship full 4x128 rows (2KB contiguous per channel); cols 125..127
and rows 125..127 carry garbage that the host zeroes.
"""

import numpy as np

import concourse.bacc as bacc
import concourse.mybir as mybir
import concourse.tile as tile
from concourse.bass_utils import run_bass_kernel_spmd

N_CORES = 8
B, CIN, H, W = 16, 128, 128, 128
COUT = 256
BPC = B // N_CORES  # images per core
HP = H + 2  # padded rows
WPAD = 128  # padded cols: -1..126 (col 127 dropped; only feeds invalid outs)
VALID = 125  # valid output rows/cols; rest is zero
ROWS_PER_TILE = 4  # output rows per PSUM tile
NMOV = 3 * WPAD + 126  # 510: covers 4 rows x 125 valid cols; must be even
F32 = mybir.dt.float32
F32R = mybir.dt.float32r

# Row-band chunks of the padded plane: (first_row_tile, n_row_tiles).
# Tiles rb0..rb0+n-1 need padded rows 4*rb0 .. 4*(rb0+n-1)+5. The first
# chunks are small so the first matmul fires as early as possible.
CHUNKS = [(0, 1), (1, 3), (4, 4), (8, 4), (12, 4), (16, 4), (20, 4), (24, 4), (28, 4)]
CHUNK_MAX_ROWS = 4 * 4 + 2  # slot size: 4 row-tiles + 2 halo rows

_NC_CACHE = {}


def _build_nc(reps=1):
    nc = bacc.Bacc()
    # Inputs are declared float32r (TF32-like: fp32 with 11 mantissa bits,
    # low 12 bits zero). Host pre-rounds, so DMA'd bytes are valid fp32r.
    # x arrives pre-padded and flattened per channel to HP*WPAD.
    x_in = nc.dram_tensor("x", [BPC, CIN, HP * WPAD], F32R, kind="ExternalInput")
    w_in = nc.dram_tensor("w", [CIN, 9 * COUT], F32R, kind="ExternalInput")
    out_t = nc.dram_tensor("out", [BPC, COUT, H * W], F32, kind="ExternalOutput")

    with tile.TileContext(nc) as tc:
        with (
            tc.tile_pool(name="wpool", bufs=1) as wpool,
            tc.tile_pool(name="xpool", bufs=4) as xpool,
            tc.tile_pool(name="opool", bufs=6) as opool,
            tc.tile_pool(name="pspool", bufs=8, space="PSUM") as pspool,
        ):
            w_sb = wpool.tile([CIN, 9 * COUT], F32R)
            nc.sync.dma_start(out=w_sb[:], in_=w_in[:])

            for b in [b for _ in range(reps) for b in range(BPC)]:
                for rb0, ntiles in CHUNKS:
                    r0 = 4 * rb0  # first padded row of the chunk
                    nrows = 4 * ntiles + 2
                    xc = xpool.tile([CIN, CHUNK_MAX_ROWS * WPAD], F32R)
                    nc.sync.dma_start(
                        out=xc[:, 0 : nrows * WPAD],
                        in_=x_in[b, :, r0 * WPAD : (r0 + nrows) * WPAD],
                    )
                    for lrb in range(ntiles):
                        r = 4 * (rb0 + lrb)  # first output row of this tile
                        lr = 4 * lrb  # row offset within the chunk
                        for c2 in range(2):
                            ps = pspool.tile([128, 512], F32)
                            for i, t in enumerate(range(9)):
                                kh, kw = divmod(t, 3)
                                c0 = t * COUT + c2 * 128
                                s = (lr + kh) * WPAD + kw
                                nc.tensor.matmul(
                                    ps[:, 0:NMOV],
                                    w_sb[:, c0 : c0 + 128],
                                    xc[:, s : s + NMOV],
                                    start=(i == 0),
                                    stop=(i == 8),
                                )
                            ob = opool.tile([128, 512], F32)
                            nc.vector.tensor_copy(
                                out=ob[:, 0:NMOV], in_=ps[:, 0:NMOV]
                            )
                            # Full-width store: 2KB contiguous per channel.
                            # Cols 125..127 (and rows 125..127 of the last
                            # tile) carry garbage; the host zeroes them.
                            nc.sync.dma_start(
                                out=out_t[
                                    b,
                                    c2 * 128 : (c2 + 1) * 128,
                                    r * W : r * W + 512,
                                ],
                                in_=ob[:],
                            )
    # Bacc defers register allocation and wait-splitting to compile(),
    # which finalize() runs; the SPMD exec path expects it done already.
    nc.finalize()
    return nc


def _get_nc(reps=1):
    if reps not in _NC_CACHE:
        _NC_CACHE[reps] = _build_nc(reps)
    return _NC_CACHE[reps]


def _round_fp32r(a):
    """Round fp32 to the hardware fp32r format: 11 mantissa bits, RNE."""
    u = np.ascontiguousarray(a, dtype=np.float32).view(np.uint32)
    r = (u + np.uint32(0x7FF) + ((u >> np.uint32(12)) & np.uint32(1))) & np.uint32(
        0xFFFFF000
    )
    return r.view(np.float32)


def _run(x, K, trace=False, reps=1):
    # Padded plane: rows -1..128, cols -1..126 (width 128). Original col
    # 127 is dropped — it only contributes to invalid output cols >= 126.
    x_pad = np.zeros((B, CIN, HP, WPAD), dtype=np.float32)
    x_pad[:, :, 1 : H + 1, 1:WPAD] = _round_fp32r(x)[:, :, :, 0 : WPAD - 1]
    x_pad = x_pad.reshape(B, CIN, HP * WPAD)
    # Reference reinterprets K's flat buffer as (Cin, kh, kw, Cout); flat
    # (128, 2304) rows are Cin, cols are (kh*3+kw)*256 + cout.
    w_host = _round_fp32r(np.asarray(K, dtype=np.float32)).reshape(CIN, 9 * COUT)
    in_maps = [
        {"x": x_pad[i * BPC : (i + 1) * BPC], "w": w_host} for i in range(N_CORES)
    ]
    res = run_bass_kernel_spmd(
        _get_nc(reps), in_maps, list(range(N_CORES)), trace=trace
    )
    out = np.concatenate(
        [res.results[i]["out"].reshape(BPC, COUT, H, W) for i in range(N_CORES)],
        axis=0,
    )
    # Device only writes valid data in the 125x125 region; zero the border
    # strips (they carry garbage from the full-width stores).
    out[:, :, VALID:, :] = 0
    out[:, :, :, VALID:] = 0
    return out, res


def kernel(x, K):
    out, _ = _run(x, K, trace=False)
    return out


# revision 5
# speedup vs baseline: 1.2322x; 1.0229x over previous
"""Trainium2 Bass kernel for a 3x3 stride-1 pad-1 Conv2d (NCHW).

Problem (hardcoded): x (16, 128, 128, 128) f32, K (3, 3, 128, 256) f32.
The reference reinterprets K's flat buffer as (Cin, kh, kw, Cout) and only
writes output rows/cols 0..124 (the rest of the 128x128 output stays zero).

Strategy: data-parallel over batch — 2 images per NeuronCore on 8 cores.
Per image, the conv is 32 output tiles of 4 rows x 125 valid cols, each
accumulated over the 9 taps into one PSUM bank per Cout half (contraction
over Cin=128 partitions): 1152 matmuls/core at N=500, streaming at the
PE roofline (~213 ns/MM measured, 1 col/cycle @ 2.4 GHz).

Operands are bf16 (rel_l2 ~2.4e-3 vs the 2e-2 gate): fp32r's weight path
adds a fixed ~20 ns/MM that bf16 avoids, and an fp32r ISA rule forbids
the odd innermost count N=500 needs. Accumulation stays fp32 in PSUM.

The activation plane is zero-padded to 130 rows x 128 cols (cols -1..126;
original col 127 only feeds invalid outputs) and streams in row-band
chunks, smallest first, so the first matmul fires as soon as the runtime
startup (~8.5 us) allows. Weights load per-tap so no matmul stalls on a
bulk transfer. Dummy matmuls on a zeroed tile warm the PE's HAM clock
gate (1.2 -> 2.4 GHz) inside the DMA shadow. Stores ship full 4x128-row
tiles (2KB contiguous per channel); cols/rows >= 125 carry garbage that
the host zeroes after the gather.
"""

import ml_dtypes
import numpy as np

import concourse.bacc as bacc
import concourse.mybir as mybir
import concourse.tile as tile
from concourse.bass_utils import run_bass_kernel_spmd

N_CORES = 8
B, CIN, H, W = 16, 128, 128, 128
COUT = 256
BPC = B // N_CORES
HP = H + 2
WPAD = 128  # padded cols -1..126 (col 127 only feeds invalid outputs)
VALID = 125
ROWS_PER_TILE = 4
F32 = mybir.dt.float32
BF16 = mybir.dt.bfloat16

CHUNKS = [(0, 1), (1, 3), (4, 4), (8, 4), (12, 4), (16, 4), (20, 4), (24, 4), (28, 4)]
CHUNK_MAX_ROWS = 4 * 4 + 2

_NC_CACHE = {}


def _build_nc(reps=1):
    nc = bacc.Bacc()
    x_in = nc.dram_tensor("x", [BPC, CIN, HP, WPAD], BF16, kind="ExternalInput")
    w_in = nc.dram_tensor("w", [CIN, 9 * COUT], BF16, kind="ExternalInput")
    out_t = nc.dram_tensor("out", [BPC, COUT, H * W], F32, kind="ExternalOutput")

    with tile.TileContext(nc) as tc:
        with (
            tc.tile_pool(name="wpool", bufs=1) as wpool,
            tc.tile_pool(name="dpool", bufs=1) as dpool,
            tc.tile_pool(name="xpool", bufs=4) as xpool,
            tc.tile_pool(name="opool", bufs=6) as opool,
            tc.tile_pool(name="pspool", bufs=7, space="PSUM") as pspool,
            tc.tile_pool(name="psdummy", bufs=1, space="PSUM") as psdummy,
        ):
            w_sb = wpool.tile([CIN, 9 * COUT], BF16)
            # Tap-0 weights land before the first x chunk; the remaining
            # taps stream per-tap behind chunk0, each arriving just ahead
            # of the matmul that needs it.
            nc.sync.dma_start(out=w_sb[:, 0:256], in_=w_in[:, 0:256])

            # PE pre-warm: the HAM clock gate keeps the PE at 1.2 GHz until
            # ~3.4us of sustained matmul activity. The first real matmul
            # can't fire until w+chunk0 land (~11us); fill that DMA shadow
            # with dummy matmuls on never-written SBUF (no deps, results
            # discarded) so the real stream starts at 2.4 GHz.
            d_x = dpool.tile([CIN, 512], BF16)
            d_ps = psdummy.tile([128, 512], F32)
            nc.vector.memset(d_x[:], 0)
            for j in range(12):
                nc.tensor.matmul(
                    d_ps[:], d_x[:, 0:128], d_x[:], start=True, stop=True
                )

            first = True
            for b in [b for _ in range(reps) for b in range(BPC)]:
                for rb0, ntiles in CHUNKS:
                    r0 = 4 * rb0
                    nrows = 4 * ntiles + 2
                    xc = xpool.tile([CIN, CHUNK_MAX_ROWS, WPAD], BF16)
                    nc.sync.dma_start(
                        out=xc[:, 0:nrows, :],
                        in_=x_in[b, :, r0 : r0 + nrows, :],
                    )
                    if first:
                        for t in range(1, 9):
                            nc.sync.dma_start(
                                out=w_sb[:, t * 256 : (t + 1) * 256],
                                in_=w_in[:, t * 256 : (t + 1) * 256],
                            )
                        first = False
                    for lrb in range(ntiles):
                        r = 4 * (rb0 + lrb)
                        lr = 4 * lrb
                        for c2 in range(2):
                            ps = pspool.tile([128, ROWS_PER_TILE, VALID], F32)
                            for i, t in enumerate(range(9)):
                                kh, kw = divmod(t, 3)
                                c0 = t * COUT + c2 * 128
                                nc.tensor.matmul(
                                    ps[:],
                                    w_sb[:, c0 : c0 + 128],
                                    xc[
                                        :,
                                        lr + kh : lr + kh + ROWS_PER_TILE,
                                        kw : kw + VALID,
                                    ],
                                    start=(i == 0),
                                    stop=(i == 8),
                                )
                            ob = opool.tile([128, ROWS_PER_TILE, W], F32)
                            nc.vector.tensor_copy(out=ob[:, :, 0:VALID], in_=ps[:])
                            nc.sync.dma_start(
                                out=out_t[
                                    b,
                                    c2 * 128 : (c2 + 1) * 128,
                                    r * W : r * W + 512,
                                ],
                                in_=ob[:],
                            )
    nc.finalize()
    return nc


def _get_nc(reps=1):
    if reps not in _NC_CACHE:
        _NC_CACHE[reps] = _build_nc(reps)
    return _NC_CACHE[reps]


def _run(x, K, trace=False, reps=1):
    x_pad = np.zeros((B, CIN, HP, WPAD), dtype=ml_dtypes.bfloat16)
    x_pad[:, :, 1 : H + 1, 1:WPAD] = np.asarray(x, dtype=np.float32)[
        :, :, :, 0 : WPAD - 1
    ].astype(ml_dtypes.bfloat16)
    w_host = (
        np.asarray(K, dtype=np.float32)
        .reshape(CIN, 9 * COUT)
        .astype(ml_dtypes.bfloat16)
    )
    in_maps = [
        {"x": x_pad[i * BPC : (i + 1) * BPC], "w": w_host} for i in range(N_CORES)
    ]
    res = run_bass_kernel_spmd(
        _get_nc(reps), in_maps, list(range(N_CORES)), trace=trace
    )
    out = np.concatenate(
        [res.results[i]["out"].reshape(BPC, COUT, H, W) for i in range(N_CORES)],
        axis=0,
    )
    out[:, :, VALID:, :] = 0
    out[:, :, :, VALID:] = 0
    return out, res


def kernel(x, K):
    out, _ = _run(x, K, trace=False)
    return out
